# revision 13
# baseline (speedup 1.0000x reference)
"""TRN2 Bass kernel v2 for DeepAveragingLSTMNetwork (8 NeuronCores, SPMD).

Strategy (data-parallel over words + lane-packed ragged schedule + fp8
DoubleRow matmuls):
  * Words (len>=2) are dealt per length level round-robin to 8 cores with
    <=1 dummy pad per level, so every core shares ONE length profile.
  * The shared profile is bin-packed into ~260 "lanes" of capacity 24
    rounds; each lane runs its words back-to-back (state zeroed at word
    switch).  Active lanes form a prefix every round with A_r ~= 260 for
    ALL 24 rounds, so every matmul streams a near-constant ~260 columns.
  * Gates are computed with fp8e4 DoubleRow matmuls (2 K-tiles per
    instruction, 0.5 cyc/col): the 6 K-planes per gate m-tile are
    (onehot, h0), (h1, h2), (h3, bias-ones) -> 3 DR matmuls.  Weights are
    scaled x64 into fp8's normal range; the activation applies 1/64.
    The bias is folded into the matmul via a rank-1 plane.
  * PSUM: two [128, 4(gate banks), 512] tiles rotate per chunk; sigmoid
    runs once over the 3 contiguous (i,f,o) banks, tanh over g.
  * h state is stored as fp8 directly in the 6-slot moving-operand
    buffers (ping-pong); c is bf16; retired words' h is stashed to a
    bf16 pool buffer, masked (dummies), reduced and AllReduced.
  * glove half: word-sharded; each core reduces its 512 words' packed
    glove rows ([128, 3, 512] bf16) on the Vector engine, AllReduced at
    kernel start so the collective fully hides under the LSTM.
  * G = char_embed @ W_ih^T is computed on device in bf16 and written
    (x64, fp8) straight into the DoubleRow weight-block tile.
"""

import sys
import time

for _p in ("/opt/trn_rl_repo",):
    if _p not in sys.path:
        sys.path.append(_p)

import numpy as np
import ml_dtypes

import concourse.bass as bass
import concourse.bacc as bacc
import concourse.mybir as mybir
import concourse.tile as tile
from concourse.bass_utils import run_bass_kernel_spmd

NCORES = 8
F32 = mybir.dt.float32
BF16 = mybir.dt.bfloat16
FP8 = mybir.dt.float8e4
NP_FP8 = ml_dtypes.float8_e4m3
NP_BF16 = ml_dtypes.bfloat16
DR = mybir.MatmulPerfMode.DoubleRow

L = 24
WSCALE = 64.0       # weight -> fp8 scale
ASCALE = 16.0       # pooled-sum -> fp8 scale (sum/16)
# torch gate order i,f,g,o ; our bank order i,f,o,g
TG = [0, 1, 3, 2]   # bank g' -> torch gate index


def _build_schedule(char_lengths):
    """Shared-profile lane packing.  Returns per-core word slots plus the
    compile-time schedule (identical across cores)."""
    lengths = np.asarray(char_lengths)
    n_idx = {}   # level -> per-core slot count
    per_core_words = {l: [[] for _ in range(NCORES)] for l in range(2, L + 1)}
    for l in range(L + 1, 1, -1):
        if l > L:
            continue
    for l in range(L, 1, -1):
        idx = np.where(lengths == l)[0]
        if len(idx) == 0:
            continue
        n_idx[l] = (len(idx) + NCORES - 1) // NCORES
        for ci in range(NCORES):
            per_core_words[l][ci] = [int(w) for w in idx[ci::NCORES]]

    # bin pack shared items (length, slot) with best-fit decreasing
    items = []
    for l, n in sorted(n_idx.items(), reverse=True):
        for k in range(n):
            items.append((l, k))
    bins = []  # list of [ (l, slot), ... ]
    for l, k in items:
        best = None
        best_rem = None
        for b in bins:
            rem = L - sum(x[0] for x in b)
            if rem >= l and (best_rem is None or rem < best_rem):
                best, best_rem = b, rem
        if best is None:
            bins.append([(l, k)])
        else:
            best.append((l, k))

    def comp(b):
        return tuple(x[0] for x in b)

    bins.sort(key=lambda b: (-sum(comp(b)), comp(b)))
    NL = len(bins)
    fills = [sum(comp(b)) for b in bins]
    A = [sum(1 for f in fills if f > r) for r in range(L)]

    # stash slots: (lane, word#) -> stash col; group switches per round
    switches = {}   # round -> list of (lo, hi, stash_lo)
    finals = []     # (buf_idx, lo, hi, stash_lo)
    stash_of = {}   # (lane, wi) -> stash col
    s = 0
    # mid-lane switches: for each distinct (composition) group and each
    # internal boundary, a contiguous lane range switches together
    gi = 0
    groups = []
    while gi < NL:
        gj = gi
        while gj < NL and comp(bins[gj]) == comp(bins[gi]):
            gj += 1
        groups.append((gi, gj))
        gi = gj
    events = []  # (round, lo, hi, wi)  wi = word index ending at round-1
    for lo, hi in groups:
        c = comp(bins[lo])
        acc = 0
        for wi, l in enumerate(c):
            acc += l
            if wi < len(c) - 1:
                events.append((acc, lo, hi, wi))
            else:
                events.append((-acc, lo, hi, wi))  # final (ends at acc)
    # assign stash columns in event order (mid switches by round, then finals)
    mids = sorted([e for e in events if e[0] > 0])
    fins = sorted([(-e[0], e[1], e[2], e[3]) for e in events if e[0] < 0])
    for r, lo, hi, wi in mids:
        switches.setdefault(r, []).append((lo, hi, s))
        for lane in range(lo, hi):
            stash_of[(lane, wi)] = s + (lane - lo)
        s += hi - lo
    for f, lo, hi, wi in fins:
        finals.append((f % 2, lo, hi, s))
        for lane in range(lo, hi):
            stash_of[(lane, wi)] = s + (lane - lo)
        s += hi - lo
    NW = s
    return dict(bins=bins, per_core_words=per_core_words, NL=NL, NW=NW,
                A=A, switches=switches, finals=finals, stash_of=stash_of)


def _ap(src, dims):
    """Rebuild an AP with an explicit [stride, size] list (partition first)."""
    return bass.AP(tensor=src.tensor, offset=src.offset, ap=dims)


def _build_program(NL, NW, A, switches, finals, VC, DC, H, DW, HID, OUT,
                   n_total, skip=()):
    SA = sum(hi - lo for evs in switches.values() for (lo, hi, _) in evs)
    H4 = 4 * H
    KH = H // 128
    DWP = 128 * ((DW + 127) // 128)           # 384
    rounds = [r for r in range(L) if A[r] > 0]

    nc = bacc.Bacc(num_devices=NCORES)

    oh_ext = nc.declare_dram_parameter("onehot", [L, 128, NL], FP8, isOutput=False)
    wp12_ext = nc.declare_dram_parameter("wp12", [128, 16, 2, 2, 128], FP8, isOutput=False)
    bias_ext = nc.declare_dram_parameter("biasp", [128, 16, 128], FP8, isOutput=False)
    cembT_ext = nc.declare_dram_parameter("cembT", [128, 128], BF16, isOutput=False)
    wih64_ext = nc.declare_dram_parameter("wih64", [128, H4], BF16, isOutput=False)
    glp_ext = nc.declare_dram_parameter("glp", [128, DWP // 128, 512], BF16, isOutput=False)
    mask_ext = nc.declare_dram_parameter("mask", [NW], BF16, isOutput=False)
    KMLP = KH + DWP // 128
    fc1t_ext = nc.declare_dram_parameter("fc1t", [128, KMLP, HID], BF16, isOutput=False)
    fc1b_ext = nc.declare_dram_parameter("fc1b", [128, HID // 128], F32, isOutput=False)
    fc2T_ext = nc.declare_dram_parameter("fc2T", [128, HID // 128, OUT], F32, isOutput=False)
    fc2b_ext = nc.declare_dram_parameter("fc2b", [OUT], F32, isOutput=False)
    out_ext = nc.declare_dram_parameter("out", [1, OUT], F32, isOutput=True)

    sc_part = nc.dram_tensor("sc_part", [H], F32)
    sc_red = nc.dram_tensor("sc_red", [H], F32, addr_space="Shared")

    Sig = mybir.ActivationFunctionType.Sigmoid
    Tanh = mybir.ActivationFunctionType.Tanh
    Copy = mybir.ActivationFunctionType.Copy
    AX = mybir.AxisListType.X
    ADD = mybir.AluOpType.add
    MUL = mybir.AluOpType.mult

    with tile.TileContext(nc) as tc:
        with (
            tc.tile_pool(name="consts", bufs=1) as consts,
            tc.tile_pool(name="ifop", bufs=2) as ifop,
            tc.tile_pool(name="gp", bufs=2) as gpool,
            tc.tile_pool(name="tcp", bufs=2) as tcp,
            tc.tile_pool(name="igp", bufs=2) as igp,
            tc.tile_pool(name="psg", bufs=2, space="PSUM") as psg,
        ):
            # ---- startup DMAs ----
            cemb_sb = consts.tile([128, 128], BF16, tag="cemb_sb")
            nc.sync.dma_start(out=cemb_sb, in_=cembT_ext[:, :])
            wih_sb = consts.tile([128, H4], BF16, tag="wih_sb")
            nc.sync.dma_start(out=wih_sb, in_=wih64_ext[:, :])

            xh0 = consts.tile([128, 6, NL], FP8, tag="xh0")
            xh1 = consts.tile([128, 6, NL], FP8, tag="xh1")
            xbufs = [xh0, xh1]
            nc.sync.dma_start(out=xh0[:, 0, :A[0]], in_=oh_ext[0, :, :A[0]])
            if len(rounds) > 1:
                nc.sync.dma_start(out=xh1[:, 0, :A[1]], in_=oh_ext[1, :, :A[1]])

            wlhs = consts.tile([128, 16, 3, 2, 128], FP8, tag="wlhs")
            nc.sync.dma_start(out=wlhs[:, :, 1:3, :, :], in_=wp12_ext[:, :, :, :, :])
            nc.sync.dma_start(out=wlhs[:, :, 0, 1, :], in_=bias_ext[:, :, :])

            glp_sb = consts.tile([128, DWP // 128, 512], BF16, tag="glp_sb")
            nc.scalar.dma_start(out=glp_sb, in_=glp_ext[:, :, :])
            fc1_sb = consts.tile([128, KMLP, HID], BF16, tag="fc1_sb")
            nc.scalar.dma_start(out=fc1_sb, in_=fc1t_ext[:, :, :])
            one_sb = consts.tile([128, 1], F32, tag="one_sb")
            nc.vector.memset(one_sb, 1.0)

            # ---- state init ----
            for xb in xbufs:
                nc.vector.memset(xb[:, 1:6, :], 0.0)
                nc.vector.memset(xb[0:1, 1, :], 1.0)
            cT = consts.tile([128, KH, NL], BF16, tag="cT")
            nc.vector.memset(cT, 0.0)
            stash = consts.tile([128, KH, NW], BF16, tag="stash")
            nc.vector.memset(stash, 0.0)

            # ---- G = cemb @ W_ih^T (bf16) -> x64 fp8 into wlhs pair0/plane0 ----
            for c in range(H4 // 512):
                gps = psg.tile([128, 4, 512], F32, tag="g")
                nc.tensor.matmul(gps[:, 0, :], cemb_sb,
                                 wih_sb[:, c * 512:(c + 1) * 512],
                                 start=True, stop=True)
                src = gps[:, 0, :]
                dst = wlhs[:, 4 * c, 0, 0, :]
                # out viewed as [128, 4 m-tiles, 128], in as [128, 4, 128]
                dst4 = _ap(dst, [list(dst.ap[0]), [3 * 2 * 128, 4], [1, 128]])
                src4 = _ap(src, [list(src.ap[0]), [128, 4], [1, 128]])
                nc.scalar.activation(dst4, src4, Copy)

            # ---- glove: reduce packed rows, AllReduce early ----
            glr = consts.tile([128, DWP // 128], F32, tag="glr")
            nc.vector.tensor_reduce(glr, glp_sb, axis=AX, op=ADD)
            glrb = consts.tile([128, DWP // 128], BF16, tag="glrb")
            nc.vector.tensor_copy(glrb, glr)
            gp = psg.tile([128, 4, 512], F32, tag="g", name="gp")
            for k in range(DWP // 128):
                nc.tensor.matmul(gp[0:1, 0, :HID], glrb[:, k:k + 1],
                                 fc1_sb[:, KH + k, :],
                                 start=(k == 0), stop=(k == DWP // 128 - 1))
            pre_g = consts.tile([128, HID], F32, tag="pre_g")
            nc.vector.tensor_copy(pre_g[0:1, :], gp[0:1, 0, :HID])

            # ---- the 24 rounds ----
            # switch handling for round r+1 is emitted during round r, split
            # per slot-pair so it chains off each pair's h-write instead of
            # serializing the whole round boundary.
            for r in rounds:
                m = A[r]
                rd = xbufs[r % 2]
                wr = xbufs[(r + 1) % 2]
                pt = [psg.tile([128, 4, 512], F32, tag="g", name=f"r{r}c{j}")
                      for j in range(KH)]
                for j in range(KH):
                    for p in range(3):
                        for g in range(4):
                            nc.tensor.matmul(
                                pt[j][:, g, :m],
                                wlhs[:, 4 * j + g, p],
                                rd[:, 2 * p:2 * p + 2, :m],
                                start=(p == 0), stop=(p == 2),
                                perf_mode=DR,
                            )
                if r + 2 in rounds:
                    nc.sync.dma_start(out=rd[:, 0, :A[r + 2]],
                                      in_=oh_ext[r + 2, :, :A[r + 2]])
                for t in range(KH // 2):   # chunk pairs (2t, 2t+1)
                    ifo = ifop.tile([128, 2, 3, NL], BF16, tag="ifo")
                    gsb = gpool.tile([128, 2, NL], BF16, tag="g_sb")
                    for jj in range(2):
                        j = 2 * t + jj
                        nc.scalar.activation(ifo[:, jj, :, :m], pt[j][:, 0:3, :m],
                                             Sig, scale=1.0 / WSCALE)
                        nc.scalar.activation(gsb[:, jj, :m], pt[j][:, 3, :m],
                                             Tanh, scale=1.0 / WSCALE)
                    ig = igp.tile([128, 2, NL], BF16, tag="ig")
                    i2 = ifo[:, :, 0, :m]
                    f2 = ifo[:, :, 1, :m]
                    o2 = ifo[:, :, 2, :m]
                    c2 = cT[:, 2 * t:2 * t + 2, :m]
                    nc.vector.tensor_tensor(ig[:, :, :m], i2, gsb[:, :, :m], op=MUL)
                    nc.vector.tensor_tensor(c2, f2, c2, op=MUL)
                    nc.vector.tensor_tensor(c2, c2, ig[:, :, :m], op=ADD)
                    tcs = tcp.tile([128, 2, NL], BF16, tag="tc")
                    nc.scalar.activation(tcs[:, :, :m], c2, Tanh)
                    nc.vector.tensor_tensor(wr[:, 2 + 2 * t:4 + 2 * t, :m], o2,
                                            tcs[:, :, :m], op=MUL)
                    # switch bookkeeping for round r+1 on these two h slots
                    for (lo, hi, st) in switches.get(r + 1, []):
                        w = hi - lo
                        nc.gpsimd.tensor_copy(stash[:, 2 * t:2 * t + 2, st:st + w],
                                              wr[:, 2 + 2 * t:4 + 2 * t, lo:hi])
                        nc.gpsimd.memset(wr[:, 2 + 2 * t:4 + 2 * t, lo:hi], 0.0)
                        nc.gpsimd.memset(cT[:, 2 * t:2 * t + 2, lo:hi], 0.0)

                if r == 22:
                    mhA = consts.tile([128, KH, NW], BF16, tag="mhA")
                    maskA = _ap(mask_sb[:, :],
                                [list(mask_sb.ap[0]), [0, KH]] + [list(d) for d in mask_sb.ap[1:]])
                    nc.vector.tensor_tensor(mhA[:, :, :SA], stash[:, :, :SA],
                                            _ap(maskA, [list(maskA.ap[0]), [0, KH], [1, SA]]),
                                            op=MUL)
                    sumA = consts.tile([128, KH], F32, tag="sumA")
                    nc.vector.tensor_reduce(sumA, mhA[:, :, :SA], axis=AX, op=ADD)
                if r == 12:
                    fc1b_sb = consts.tile([128, HID // 128], F32, tag="fc1b_sb")
                    nc.scalar.dma_start(out=fc1b_sb, in_=fc1b_ext[:, :])
                    fc2_sb = consts.tile([128, HID // 128, OUT], F32, tag="fc2_sb")
                    nc.scalar.dma_start(out=fc2_sb, in_=fc2T_ext[:, :, :])
                    fc2b_sb = consts.tile([128, 1], F32, tag="fc2b_sb")
                    nc.scalar.dma_start(out=fc2b_sb[:OUT, 0], in_=fc2b_ext[:])
                    mask_sb = consts.tile([128, NW], BF16, tag="mask_sb")
                    m_ap = mask_ext[:]
                    nc.scalar.dma_start(
                        out=mask_sb,
                        in_=bass.AP(tensor=m_ap.tensor, offset=m_ap.offset,
                                    ap=[[0, 128]] + list(m_ap.ap)),
                    )

            # ---- final stashes ----
            for (buf, lo, hi, st) in finals:
                w = hi - lo
                nc.vector.tensor_copy(stash[:, :, st:st + w],
                                      xbufs[buf][:, 2:6, lo:hi])

            # ---- masked pooled char sum -> char half of fc1 preact ----
            NB = NW - SA
            mh = consts.tile([128, KH, NW], BF16, tag="mh")
            mask4 = _ap(mask_sb[:, :],
                        [list(mask_sb.ap[0]), [0, KH]] + [list(d) for d in mask_sb.ap[1:]])
            mh_b = mh[:, :, SA:]
            maskB = _ap(mask_sb[:, SA:],
                        [list(mask_sb.ap[0]), [0, KH], [1, NB]])
            nc.vector.tensor_tensor(mh[:, :, SA:], stash[:, :, SA:], maskB, op=MUL)
            sum4f = consts.tile([128, KH], F32, tag="sum4f")
            nc.vector.tensor_reduce(sum4f, mh[:, :, SA:], axis=AX, op=ADD)
            nc.vector.tensor_tensor(sum4f, sum4f, sumA, op=ADD)
            sum4 = consts.tile([128, KH], BF16, tag="sum4")
            nc.vector.tensor_copy(sum4, sum4f)
            cp = psg.tile([128, 4, 512], F32, tag="g", name="cp")
            for k in range(KH):
                nc.tensor.matmul(cp[0:1, 0, :HID], sum4[:, k:k + 1],
                                 fc1_sb[:, k, :],
                                 start=(k == 0), stop=(k == KH - 1))
            pr_sb = consts.tile([128, HID], F32, tag="pr_sb")
            nc.vector.tensor_tensor(pr_sb[0:1, :], cp[0:1, 0, :HID],
                                    pre_g[0:1, :], op=ADD)
            # transpose the preact row to partition-major via K=1 matmuls
            pcp = psg.tile([128, 4, 512], F32, tag="g", name="pcp")
            for i in range(HID // 128):
                nc.tensor.matmul(pcp[:, 0, i:i + 1],
                                 pr_sb[0:1, i * 128:(i + 1) * 128],
                                 one_sb[0:1, 0:1], start=True, stop=True)
            pc4 = consts.tile([128, HID // 128], F32, tag="pc4")
            nc.vector.tensor_copy(pc4, pcp[:, 0, 0:HID // 128])
            sc_pm = sc_part.rearrange("(p k) -> p k", k=KH)
            nc.scalar.dma_start(out=sc_pm, in_=pc4)
            if "coll" in skip:
                nc.scalar.dma_start(out=sc_red[:], in_=sc_part[:])
            else:
                nc.gpsimd.collective_compute(
                    "AllReduce", ADD,
                    replica_groups=[list(range(NCORES))],
                    ins=[sc_part[:]], outs=[sc_red[:]],
                )

            # ---- head tail: sigmoid + fc2 (identical on every core) ----
            pc_sb = consts.tile([128, HID // 128], F32, tag="pc_sb")
            nc.scalar.dma_start(out=pc_sb,
                                in_=sc_red.rearrange("(p k) -> p k", k=KH))
            h1_sb = consts.tile([128, HID // 128], F32, tag="h1_sb")
            hscale = 1.0 / float(n_total)
            for i in range(HID // 128):
                nc.scalar.activation(h1_sb[:, i:i + 1], pc_sb[:, i:i + 1], Sig,
                                     bias=fc1b_sb[:, i:i + 1], scale=hscale)
            lp = psg.tile([128, 4, 512], F32, tag="g", name="lp")
            for k in range(HID // 128):
                nc.tensor.matmul(lp[:OUT, 0, 0:1], fc2_sb[:, k, :],
                                 h1_sb[:, k:k + 1],
                                 start=(k == 0), stop=(k == HID // 128 - 1))
            lo_sb = consts.tile([128, 1], F32, tag="lo_sb")
            nc.vector.tensor_tensor(lo_sb[:OUT, :], lp[:OUT, 0, 0:1],
                                    fc2b_sb[:OUT, :], op=ADD)
            nc.scalar.dma_start(out=out_ext[0, :], in_=lo_sb[:OUT, 0])

    nc.compile()
    return nc


def _prep_inputs(inputs, sched):
    """Host prep: index re-encodings + dtype packing. Returns in_maps."""
    word_indices = np.asarray(inputs["word_indices"])
    char_indices = np.asarray(inputs["char_indices"])
    glove_table = np.asarray(inputs["glove_table"], dtype=np.float32)
    char_embed = np.asarray(inputs["char_embed"], dtype=np.float32)
    W_ih = np.asarray(inputs["W_ih"], dtype=np.float32)
    W_hh = np.asarray(inputs["W_hh"], dtype=np.float32)
    b_ih = np.asarray(inputs["b_ih"], dtype=np.float32)
    b_hh = np.asarray(inputs["b_hh"], dtype=np.float32)
    fc1_W = np.asarray(inputs["fc1_W"], dtype=np.float32)
    fc1_b = np.asarray(inputs["fc1_b"], dtype=np.float32)
    fc2_W = np.asarray(inputs["fc2_W"], dtype=np.float32)
    fc2_b = np.asarray(inputs["fc2_b"], dtype=np.float32)

    N = word_indices.shape[0]
    VW, DW = glove_table.shape
    VC, DC = char_embed.shape
    H = W_hh.shape[1]
    H4 = 4 * H
    HID, OUT = fc1_W.shape[0], fc2_W.shape[0]
    DWP = 128 * ((DW + 127) // 128)
    NL, NW = sched["NL"], sched["NW"]
    bins, stash_of = sched["bins"], sched["stash_of"]
    per_core_words = sched["per_core_words"]
    b = b_ih + b_hh

    # --- shared (replicated) weight blocks ---
    # permuted gate rows: m-tile mt = 4*j + g' covers torch rows
    # TG[g']*H + j*128 .. +128
    def rows(mt):
        j, g = mt // 4, mt % 4
        lo = TG[g] * H + j * 128
        return slice(lo, lo + 128)

    whhT = W_hh.T  # [H, 4H]
    wp12 = np.zeros((128, 16, 2, 2, 128), np.float32)
    biasp = np.zeros((128, 16, 128), np.float32)
    for mt in range(16):
        blk = whhT[:, rows(mt)] * WSCALE            # [512, 128]
        wp12[:, mt, 0, 0, :] = blk[0:128]
        wp12[:, mt, 0, 1, :] = blk[128:256]
        wp12[:, mt, 1, 0, :] = blk[256:384]
        wp12[:, mt, 1, 1, :] = blk[384:512]
        biasp[0, mt, :] = b[rows(mt)] * WSCALE
    wp12 = wp12.astype(NP_FP8)
    biasp = biasp.astype(NP_FP8)

    cembT = np.zeros((128, 128), np.float32)
    cembT[:DC, :VC] = char_embed.T
    wih64 = np.zeros((128, H4), np.float32)
    wihT = W_ih.T * WSCALE                          # [DC, 4H]
    for mt in range(16):
        wih64[:DC, mt * 128:(mt + 1) * 128] = wihT[:, rows(mt)]

    KMLP = H // 128 + DWP // 128
    fc1t = np.zeros((128, KMLP, HID), np.float32)
    # avg planes: 0..3 char (H), 4..6 glove (DWP)
    fc1c = fc1_W[:, DW:].T                          # [H, HID]
    fc1g = fc1_W[:, :DW].T                          # [DW, HID]
    for k in range(H // 128):
        fc1t[:, k, :] = fc1c[k * 128:(k + 1) * 128]
    for k in range(DWP // 128):
        blk = fc1g[k * 128:min((k + 1) * 128, DW)]
        fc1t[:blk.shape[0], H // 128 + k, :] = blk

    shared = dict(
        wp12=wp12, biasp=biasp,
        cembT=cembT.astype(NP_BF16),
        wih64=wih64.astype(NP_BF16),
        fc1t=fc1t.astype(NP_BF16),
        fc1b=np.ascontiguousarray(fc1_b.reshape(-1, 128).T),
        fc2T=np.ascontiguousarray(fc2_W.T.reshape(-1, 128, OUT).transpose(1, 0, 2)),
        fc2b=fc2_b,
    )

    # --- per-core tensors ---
    rows_per = N // NCORES
    in_maps = []
    for ci in range(NCORES):
        # lane word assignment for this core; -1 = dummy
        lane_words = []
        for bwords in bins:
            seq = []
            for (l, k) in bwords:
                wl = per_core_words[l][ci]
                seq.append((wl[k] if k < len(wl) else -1, l))
            lane_words.append(seq)

        oh = np.zeros((L, 128, NL), NP_FP8)
        mask = np.zeros(NW, NP_BF16)
        for lane, seq in enumerate(lane_words):
            r0 = 0
            for wi, (w, l) in enumerate(seq):
                if w >= 0:
                    mask[stash_of[(lane, wi)]] = 1.0
                    for t in range(l):
                        oh[r0 + t, char_indices[w, t], lane] = 1.0
                r0 += l

        # glove rows packed transposed: [128, DWP//128, 512]
        wids = word_indices[ci * rows_per:(ci + 1) * rows_per]
        gl = np.zeros((512, DWP), np.float32)
        gl[:len(wids), :DW] = glove_table[wids]
        glp = np.ascontiguousarray(
            gl.T.reshape(DWP // 128, 128, 512).transpose(1, 0, 2))

        in_maps.append(dict(
            onehot=oh,
            mask=mask,
            glp=glp.astype(NP_BF16),
            **shared,
        ))
    return in_maps


def kernel(**inputs):
    char_lengths = np.asarray(inputs["char_lengths"])
    char_indices = np.asarray(inputs["char_indices"])
    N = char_indices.shape[0]
    VW, DW = np.asarray(inputs["glove_table"]).shape
    VC, DC = np.asarray(inputs["char_embed"]).shape
    H = np.asarray(inputs["W_hh"]).shape[1]
    HID = np.asarray(inputs["fc1_W"]).shape[0]
    OUT = np.asarray(inputs["fc2_W"]).shape[0]

    sched = _build_schedule(char_lengths)
    assert sched["NL"] <= 512, f"lane count {sched['NL']} exceeds PSUM bank"

    nc = _build_program(sched["NL"], sched["NW"], sched["A"],
                        sched["switches"], sched["finals"],
                        VC, DC, H, DW, HID, OUT, N)
    in_maps = _prep_inputs(inputs, sched)

    res = None
    for attempt in range(3):
        try:
            res = run_bass_kernel_spmd(nc, in_maps, list(range(NCORES)))
            break
        except Exception:
            if attempt == 2:
                raise
            time.sleep(2.0)
    global _LAST_RESULTS
    _LAST_RESULTS = res
    return np.array(res.results[0]["out"], dtype=np.float32)


_LAST_RESULTS = None


# revision 14
# speedup vs baseline: 1.5487x; 1.5487x over previous
"""TRN2 Bass kernel v2 for DeepAveragingLSTMNetwork (8 NeuronCores, SPMD).

Strategy (data-parallel over words + lane-packed ragged schedule + fp8
DoubleRow matmuls):
  * Words (len>=2) are dealt per length level round-robin to 8 cores with
    <=1 dummy pad per level, so every core shares ONE length profile.
  * The shared profile is bin-packed into ~260 "lanes" of capacity 24
    rounds; each lane runs its words back-to-back (state zeroed at word
    switch).  Active lanes form a prefix every round with A_r ~= 260 for
    ALL 24 rounds, so every matmul streams a near-constant ~260 columns.
  * Gates are computed with fp8e4 DoubleRow matmuls (2 K-tiles per
    instruction, 0.5 cyc/col): the 6 K-planes per gate m-tile are
    (onehot, h0), (h1, h2), (h3, bias-ones) -> 3 DR matmuls.  Weights are
    scaled x64 into fp8's normal range; the activation applies 1/64.
    The bias is folded into the matmul via a rank-1 plane.
  * PSUM: two [128, 4(gate banks), 512] tiles rotate per chunk; sigmoid
    runs once over the 3 contiguous (i,f,o) banks, tanh over g.
  * h state is stored as fp8 directly in the 6-slot moving-operand
    buffers (ping-pong); c is bf16; retired words' h is stashed to a
    bf16 pool buffer, masked (dummies), reduced and AllReduced.
  * glove half: word-sharded; each core reduces its 512 words' packed
    glove rows ([128, 3, 512] bf16) on the Vector engine, AllReduced at
    kernel start so the collective fully hides under the LSTM.
  * G = char_embed @ W_ih^T is computed on device in bf16 and written
    (x64, fp8) straight into the DoubleRow weight-block tile.
"""

import sys
import time

for _p in ("/opt/trn_rl_repo",):
    if _p not in sys.path:
        sys.path.append(_p)

import numpy as np
import ml_dtypes

import concourse.bass as bass
import concourse.bacc as bacc
import concourse.mybir as mybir
import concourse.tile as tile
from concourse.bass_utils import run_bass_kernel_spmd

NCORES = 8
F32 = mybir.dt.float32
BF16 = mybir.dt.bfloat16
FP8 = mybir.dt.float8e4
NP_FP8 = ml_dtypes.float8_e4m3
NP_BF16 = ml_dtypes.bfloat16
DR = mybir.MatmulPerfMode.DoubleRow

L = 24
WSCALE = 64.0       # weight -> fp8 scale
ASCALE = 16.0       # pooled-sum -> fp8 scale (sum/16)
# torch gate order i,f,g,o ; our bank order i,f,o,g
TG = [0, 1, 3, 2]   # bank g' -> torch gate index


def _build_schedule(char_lengths):
    """Shared-profile lane packing.  Returns per-core word slots plus the
    compile-time schedule (identical across cores)."""
    lengths = np.asarray(char_lengths)
    n_idx = {}   # level -> per-core slot count
    per_core_words = {l: [[] for _ in range(NCORES)] for l in range(2, L + 1)}
    for l in range(L + 1, 1, -1):
        if l > L:
            continue
    for l in range(L, 1, -1):
        idx = np.where(lengths == l)[0]
        if len(idx) == 0:
            continue
        n_idx[l] = (len(idx) + NCORES - 1) // NCORES
        for ci in range(NCORES):
            per_core_words[l][ci] = [int(w) for w in idx[ci::NCORES]]

    # bin pack shared items (length, slot) with best-fit decreasing
    items = []
    for l, n in sorted(n_idx.items(), reverse=True):
        for k in range(n):
            items.append((l, k))
    bins = []  # list of [ (l, slot), ... ]
    for l, k in items:
        best = None
        best_rem = None
        for b in bins:
            rem = L - sum(x[0] for x in b)
            if rem >= l and (best_rem is None or rem < best_rem):
                best, best_rem = b, rem
        if best is None:
            bins.append([(l, k)])
        else:
            best.append((l, k))

    def comp(b):
        return tuple(x[0] for x in b)

    bins.sort(key=lambda b: (-sum(comp(b)), comp(b)))
    NL = len(bins)
    fills = [sum(comp(b)) for b in bins]
    A = [sum(1 for f in fills if f > r) for r in range(L)]

    # stash slots: (lane, word#) -> stash col; group switches per round
    switches = {}   # round -> list of (lo, hi, stash_lo)
    finals = []     # (buf_idx, lo, hi, stash_lo)
    stash_of = {}   # (lane, wi) -> stash col
    s = 0
    # mid-lane switches: for each distinct (composition) group and each
    # internal boundary, a contiguous lane range switches together
    gi = 0
    groups = []
    while gi < NL:
        gj = gi
        while gj < NL and comp(bins[gj]) == comp(bins[gi]):
            gj += 1
        groups.append((gi, gj))
        gi = gj
    events = []  # (round, lo, hi, wi)  wi = word index ending at round-1
    for lo, hi in groups:
        c = comp(bins[lo])
        acc = 0
        for wi, l in enumerate(c):
            acc += l
            if wi < len(c) - 1:
                events.append((acc, lo, hi, wi))
            else:
                events.append((-acc, lo, hi, wi))  # final (ends at acc)
    # assign stash columns in event order (mid switches by round, then finals)
    mids = sorted([e for e in events if e[0] > 0])
    fins = sorted([(-e[0], e[1], e[2], e[3]) for e in events if e[0] < 0])
    for r, lo, hi, wi in mids:
        switches.setdefault(r, []).append((lo, hi, s))
        for lane in range(lo, hi):
            stash_of[(lane, wi)] = s + (lane - lo)
        s += hi - lo
    for f, lo, hi, wi in fins:
        finals.append((f % 2, lo, hi, s))
        for lane in range(lo, hi):
            stash_of[(lane, wi)] = s + (lane - lo)
        s += hi - lo
    NW = s
    return dict(bins=bins, per_core_words=per_core_words, NL=NL, NW=NW,
                A=A, switches=switches, finals=finals, stash_of=stash_of)


def _ap(src, dims):
    """Rebuild an AP with an explicit [stride, size] list (partition first)."""
    return bass.AP(tensor=src.tensor, offset=src.offset, ap=dims)


def _build_program(NL, NW, A, switches, finals, VC, DC, H, DW, HID, OUT,
                   n_total, skip=()):
    SA = sum(hi - lo for evs in switches.values() for (lo, hi, _) in evs)
    H4 = 4 * H
    KH = H // 128
    DWP = 128 * ((DW + 127) // 128)           # 384
    rounds = [r for r in range(L) if A[r] > 0]

    nc = bacc.Bacc(num_devices=NCORES)

    oh_ext = nc.declare_dram_parameter("onehot", [L, 128, NL], FP8, isOutput=False)
    wp12_ext = nc.declare_dram_parameter("wp12", [128, 16, 2, 2, 128], FP8, isOutput=False)
    bias_ext = nc.declare_dram_parameter("biasp", [128, 16, 128], FP8, isOutput=False)
    cembT_ext = nc.declare_dram_parameter("cembT", [128, 128], BF16, isOutput=False)
    wih64_ext = nc.declare_dram_parameter("wih64", [128, H4], BF16, isOutput=False)
    glp_ext = nc.declare_dram_parameter("glp", [128, DWP // 128, 512], BF16, isOutput=False)
    mask_ext = nc.declare_dram_parameter("mask", [NW], BF16, isOutput=False)
    KMLP = KH + DWP // 128
    fc1t_ext = nc.declare_dram_parameter("fc1t", [128, KMLP, HID], BF16, isOutput=False)
    fc1b_ext = nc.declare_dram_parameter("fc1b", [128, HID // 128], F32, isOutput=False)
    fc2T_ext = nc.declare_dram_parameter("fc2T", [128, HID // 128, OUT], F32, isOutput=False)
    fc2b_ext = nc.declare_dram_parameter("fc2b", [OUT], F32, isOutput=False)
    out_ext = nc.declare_dram_parameter("out", [1, OUT], F32, isOutput=True)

    sc_part = nc.dram_tensor("sc_part", [H], F32)
    sc_red = nc.dram_tensor("sc_red", [H], F32, addr_space="Shared")

    Sig = mybir.ActivationFunctionType.Sigmoid
    Tanh = mybir.ActivationFunctionType.Tanh
    Copy = mybir.ActivationFunctionType.Copy
    AX = mybir.AxisListType.X
    ADD = mybir.AluOpType.add
    MUL = mybir.AluOpType.mult

    with tile.TileContext(nc) as tc:
        with (
            tc.tile_pool(name="consts", bufs=1) as consts,
            tc.tile_pool(name="ifop", bufs=2) as ifop,
            tc.tile_pool(name="gp", bufs=2) as gpool,
            tc.tile_pool(name="tcp", bufs=2) as tcp,
            tc.tile_pool(name="igp", bufs=2) as igp,
            tc.tile_pool(name="psg", bufs=2, space="PSUM") as psg,
        ):
            # ---- startup DMAs ----
            cemb_sb = consts.tile([128, 128], BF16, tag="cemb_sb")
            nc.sync.dma_start(out=cemb_sb, in_=cembT_ext[:, :])
            wih_sb = consts.tile([128, H4], BF16, tag="wih_sb")
            nc.sync.dma_start(out=wih_sb, in_=wih64_ext[:, :])

            xh0 = consts.tile([128, 6, NL], FP8, tag="xh0")
            xh1 = consts.tile([128, 6, NL], FP8, tag="xh1")
            xbufs = [xh0, xh1]
            nc.sync.dma_start(out=xh0[:, 0, :A[0]], in_=oh_ext[0, :, :A[0]])
            if len(rounds) > 1:
                nc.sync.dma_start(out=xh1[:, 0, :A[1]], in_=oh_ext[1, :, :A[1]])

            wlhs = consts.tile([128, 16, 3, 2, 128], FP8, tag="wlhs")
            nc.sync.dma_start(out=wlhs[:, :, 1:3, :, :], in_=wp12_ext[:, :, :, :, :])
            nc.sync.dma_start(out=wlhs[:, :, 0, 1, :], in_=bias_ext[:, :, :])

            glp_sb = consts.tile([128, DWP // 128, 512], BF16, tag="glp_sb")
            nc.scalar.dma_start(out=glp_sb, in_=glp_ext[:, :, :])
            fc1_sb = consts.tile([128, KMLP, HID], BF16, tag="fc1_sb")
            nc.scalar.dma_start(out=fc1_sb, in_=fc1t_ext[:, :, :])
            one_sb = consts.tile([128, 1], F32, tag="one_sb")
            nc.vector.memset(one_sb, 1.0)

            # ---- state init ----
            for xb in xbufs:
                nc.vector.memset(xb[:, 1:6, :], 0.0)
                nc.vector.memset(xb[0:1, 1, :], 1.0)
            cT = consts.tile([128, KH, NL], BF16, tag="cT")
            nc.vector.memset(cT, 0.0)
            stash = consts.tile([128, KH, NW], BF16, tag="stash")
            nc.vector.memset(stash, 0.0)

            # ---- G = cemb @ W_ih^T (bf16) -> x64 fp8 into wlhs pair0/plane0 ----
            for c in range(H4 // 512):
                gps = psg.tile([128, 4, 512], F32, tag="g")
                nc.tensor.matmul(gps[:, 0, :], cemb_sb,
                                 wih_sb[:, c * 512:(c + 1) * 512],
                                 start=True, stop=True)
                src = gps[:, 0, :]
                dst = wlhs[:, 4 * c, 0, 0, :]
                # out viewed as [128, 4 m-tiles, 128], in as [128, 4, 128]
                dst4 = _ap(dst, [list(dst.ap[0]), [3 * 2 * 128, 4], [1, 128]])
                src4 = _ap(src, [list(src.ap[0]), [128, 4], [1, 128]])
                nc.scalar.activation(dst4, src4, Copy)

            # ---- glove: reduce packed rows, AllReduce early ----
            glr = consts.tile([128, DWP // 128], F32, tag="glr")
            nc.vector.tensor_reduce(glr, glp_sb, axis=AX, op=ADD)
            glrb = consts.tile([128, DWP // 128], BF16, tag="glrb")
            nc.vector.tensor_copy(glrb, glr)
            gp = psg.tile([128, 4, 512], F32, tag="g", name="gp")
            for k in range(DWP // 128):
                nc.tensor.matmul(gp[0:1, 0, :HID], glrb[:, k:k + 1],
                                 fc1_sb[:, KH + k, :],
                                 start=(k == 0), stop=(k == DWP // 128 - 1))
            pre_g = consts.tile([128, HID], F32, tag="pre_g")
            nc.vector.tensor_copy(pre_g[0:1, :], gp[0:1, 0, :HID])

            # ---- the 24 rounds ----
            # switch handling for round r+1 is emitted during round r, split
            # per slot-pair so it chains off each pair's h-write instead of
            # serializing the whole round boundary.
            for r in rounds:
                m = A[r]
                rd = xbufs[r % 2]
                wr = xbufs[(r + 1) % 2]
                pt = [psg.tile([128, 4, 512], F32, tag="g", name=f"r{r}c{j}")
                      for j in range(KH)]
                for j in range(KH):
                    for p in range(3):
                        for g in range(4):
                            nc.tensor.matmul(
                                pt[j][:, g, :m],
                                wlhs[:, 4 * j + g, p],
                                rd[:, 2 * p:2 * p + 2, :m],
                                start=(p == 0), stop=(p == 2),
                                perf_mode=DR,
                            )
                if r + 2 in rounds:
                    nc.sync.dma_start(out=rd[:, 0, :A[r + 2]],
                                      in_=oh_ext[r + 2, :, :A[r + 2]])
                for t in range(KH // 2):   # chunk pairs (2t, 2t+1)
                    ifo = ifop.tile([128, 2, 3, NL], BF16, tag="ifo")
                    gsb = gpool.tile([128, 2, NL], BF16, tag="g_sb")
                    for jj in range(2):
                        j = 2 * t + jj
                        nc.scalar.activation(ifo[:, jj, :, :m], pt[j][:, 0:3, :m],
                                             Sig, scale=1.0 / WSCALE)
                        nc.scalar.activation(gsb[:, jj, :m], pt[j][:, 3, :m],
                                             Tanh, scale=1.0 / WSCALE)
                    ig = igp.tile([128, 2, NL], BF16, tag="ig")
                    i2 = ifo[:, :, 0, :m]
                    f2 = ifo[:, :, 1, :m]
                    o2 = ifo[:, :, 2, :m]
                    c2 = cT[:, 2 * t:2 * t + 2, :m]
                    nc.vector.tensor_tensor(ig[:, :, :m], i2, gsb[:, :, :m], op=MUL)
                    nc.vector.tensor_tensor(c2, f2, c2, op=MUL)
                    nc.vector.tensor_tensor(c2, c2, ig[:, :, :m], op=ADD)
                    tcs = tcp.tile([128, 2, NL], BF16, tag="tc")
                    nc.scalar.activation(tcs[:, :, :m], c2, Tanh)
                    nc.vector.tensor_tensor(wr[:, 2 + 2 * t:4 + 2 * t, :m], o2,
                                            tcs[:, :, :m], op=MUL)
                    # switch bookkeeping for round r+1 on these two h slots
                    for (lo, hi, st) in switches.get(r + 1, []):
                        w = hi - lo
                        nc.vector.tensor_copy(stash[:, 2 * t:2 * t + 2, st:st + w],
                                              wr[:, 2 + 2 * t:4 + 2 * t, lo:hi])
                        nc.vector.memset(wr[:, 2 + 2 * t:4 + 2 * t, lo:hi], 0.0)
                        nc.vector.memset(cT[:, 2 * t:2 * t + 2, lo:hi], 0.0)

                if r == 22:
                    mhA = consts.tile([128, KH, NW], BF16, tag="mhA")
                    maskA = _ap(mask_sb[:, :],
                                [list(mask_sb.ap[0]), [0, KH]] + [list(d) for d in mask_sb.ap[1:]])
                    nc.vector.tensor_tensor(mhA[:, :, :SA], stash[:, :, :SA],
                                            _ap(maskA, [list(maskA.ap[0]), [0, KH], [1, SA]]),
                                            op=MUL)
                    sumA = consts.tile([128, KH], F32, tag="sumA")
                    nc.vector.tensor_reduce(sumA, mhA[:, :, :SA], axis=AX, op=ADD)
                if r == 12:
                    fc1b_sb = consts.tile([128, HID // 128], F32, tag="fc1b_sb")
                    nc.scalar.dma_start(out=fc1b_sb, in_=fc1b_ext[:, :])
                    fc2_sb = consts.tile([128, HID // 128, OUT], F32, tag="fc2_sb")
                    nc.scalar.dma_start(out=fc2_sb, in_=fc2T_ext[:, :, :])
                    fc2b_sb = consts.tile([128, 1], F32, tag="fc2b_sb")
                    nc.scalar.dma_start(out=fc2b_sb[:OUT, 0], in_=fc2b_ext[:])
                    mask_sb = consts.tile([128, NW], BF16, tag="mask_sb")
                    m_ap = mask_ext[:]
                    nc.scalar.dma_start(
                        out=mask_sb,
                        in_=bass.AP(tensor=m_ap.tensor, offset=m_ap.offset,
                                    ap=[[0, 128]] + list(m_ap.ap)),
                    )

            # ---- final stashes ----
            for (buf, lo, hi, st) in finals:
                w = hi - lo
                nc.vector.tensor_copy(stash[:, :, st:st + w],
                                      xbufs[buf][:, 2:6, lo:hi])

            # ---- masked pooled char sum -> char half of fc1 preact ----
            NB = NW - SA
            mh = consts.tile([128, KH, NW], BF16, tag="mh")
            mask4 = _ap(mask_sb[:, :],
                        [list(mask_sb.ap[0]), [0, KH]] + [list(d) for d in mask_sb.ap[1:]])
            mh_b = mh[:, :, SA:]
            maskB = _ap(mask_sb[:, SA:],
                        [list(mask_sb.ap[0]), [0, KH], [1, NB]])
            nc.vector.tensor_tensor(mh[:, :, SA:], stash[:, :, SA:], maskB, op=MUL)
            sum4f = consts.tile([128, KH], F32, tag="sum4f")
            nc.vector.tensor_reduce(sum4f, mh[:, :, SA:], axis=AX, op=ADD)
            nc.vector.tensor_tensor(sum4f, sum4f, sumA, op=ADD)
            sum4 = consts.tile([128, KH], BF16, tag="sum4")
            nc.vector.tensor_copy(sum4, sum4f)
            cp = psg.tile([128, 4, 512], F32, tag="g", name="cp")
            for k in range(KH):
                nc.tensor.matmul(cp[0:1, 0, :HID], sum4[:, k:k + 1],
                                 fc1_sb[:, k, :],
                                 start=(k == 0), stop=(k == KH - 1))
            pr_sb = consts.tile([128, HID], F32, tag="pr_sb")
            nc.vector.tensor_tensor(pr_sb[0:1, :], cp[0:1, 0, :HID],
                                    pre_g[0:1, :], op=ADD)
            # transpose the preact row to partition-major via K=1 matmuls
            pcp = psg.tile([128, 4, 512], F32, tag="g", name="pcp")
            for i in range(HID // 128):
                nc.tensor.matmul(pcp[:, 0, i:i + 1],
                                 pr_sb[0:1, i * 128:(i + 1) * 128],
                                 one_sb[0:1, 0:1], start=True, stop=True)
            pc4 = consts.tile([128, HID // 128], F32, tag="pc4")
            nc.vector.tensor_copy(pc4, pcp[:, 0, 0:HID // 128])
            sc_pm = sc_part.rearrange("(p k) -> p k", k=KH)
            nc.scalar.dma_start(out=sc_pm, in_=pc4)
            if "coll" in skip:
                nc.scalar.dma_start(out=sc_red[:], in_=sc_part[:])
            else:
                nc.gpsimd.collective_compute(
                    "AllReduce", ADD,
                    replica_groups=[list(range(NCORES))],
                    ins=[sc_part[:]], outs=[sc_red[:]],
                )

            # ---- head tail: sigmoid + fc2 (identical on every core) ----
            pc_sb = consts.tile([128, HID // 128], F32, tag="pc_sb")
            nc.scalar.dma_start(out=pc_sb,
                                in_=sc_red.rearrange("(p k) -> p k", k=KH))
            h1_sb = consts.tile([128, HID // 128], F32, tag="h1_sb")
            hscale = 1.0 / float(n_total)
            for i in range(HID // 128):
                nc.scalar.activation(h1_sb[:, i:i + 1], pc_sb[:, i:i + 1], Sig,
                                     bias=fc1b_sb[:, i:i + 1], scale=hscale)
            lp = psg.tile([128, 4, 512], F32, tag="g", name="lp")
            for k in range(HID // 128):
                nc.tensor.matmul(lp[:OUT, 0, 0:1], fc2_sb[:, k, :],
                                 h1_sb[:, k:k + 1],
                                 start=(k == 0), stop=(k == HID // 128 - 1))
            lo_sb = consts.tile([128, 1], F32, tag="lo_sb")
            nc.vector.tensor_tensor(lo_sb[:OUT, :], lp[:OUT, 0, 0:1],
                                    fc2b_sb[:OUT, :], op=ADD)
            nc.scalar.dma_start(out=out_ext[0, :], in_=lo_sb[:OUT, 0])

    nc.compile()
    return nc


def _prep_inputs(inputs, sched):
    """Host prep: index re-encodings + dtype packing. Returns in_maps."""
    word_indices = np.asarray(inputs["word_indices"])
    char_indices = np.asarray(inputs["char_indices"])
    glove_table = np.asarray(inputs["glove_table"], dtype=np.float32)
    char_embed = np.asarray(inputs["char_embed"], dtype=np.float32)
    W_ih = np.asarray(inputs["W_ih"], dtype=np.float32)
    W_hh = np.asarray(inputs["W_hh"], dtype=np.float32)
    b_ih = np.asarray(inputs["b_ih"], dtype=np.float32)
    b_hh = np.asarray(inputs["b_hh"], dtype=np.float32)
    fc1_W = np.asarray(inputs["fc1_W"], dtype=np.float32)
    fc1_b = np.asarray(inputs["fc1_b"], dtype=np.float32)
    fc2_W = np.asarray(inputs["fc2_W"], dtype=np.float32)
    fc2_b = np.asarray(inputs["fc2_b"], dtype=np.float32)

    N = word_indices.shape[0]
    VW, DW = glove_table.shape
    VC, DC = char_embed.shape
    H = W_hh.shape[1]
    H4 = 4 * H
    HID, OUT = fc1_W.shape[0], fc2_W.shape[0]
    DWP = 128 * ((DW + 127) // 128)
    NL, NW = sched["NL"], sched["NW"]
    bins, stash_of = sched["bins"], sched["stash_of"]
    per_core_words = sched["per_core_words"]
    b = b_ih + b_hh

    # --- shared (replicated) weight blocks ---
    # permuted gate rows: m-tile mt = 4*j + g' covers torch rows
    # TG[g']*H + j*128 .. +128
    def rows(mt):
        j, g = mt // 4, mt % 4
        lo = TG[g] * H + j * 128
        return slice(lo, lo + 128)

    whhT = W_hh.T  # [H, 4H]
    wp12 = np.zeros((128, 16, 2, 2, 128), np.float32)
    biasp = np.zeros((128, 16, 128), np.float32)
    for mt in range(16):
        blk = whhT[:, rows(mt)] * WSCALE            # [512, 128]
        wp12[:, mt, 0, 0, :] = blk[0:128]
        wp12[:, mt, 0, 1, :] = blk[128:256]
        wp12[:, mt, 1, 0, :] = blk[256:384]
        wp12[:, mt, 1, 1, :] = blk[384:512]
        biasp[0, mt, :] = b[rows(mt)] * WSCALE
    wp12 = wp12.astype(NP_FP8)
    biasp = biasp.astype(NP_FP8)

    cembT = np.zeros((128, 128), np.float32)
    cembT[:DC, :VC] = char_embed.T
    wih64 = np.zeros((128, H4), np.float32)
    wihT = W_ih.T * WSCALE                          # [DC, 4H]
    for mt in range(16):
        wih64[:DC, mt * 128:(mt + 1) * 128] = wihT[:, rows(mt)]

    KMLP = H // 128 + DWP // 128
    fc1t = np.zeros((128, KMLP, HID), np.float32)
    # avg planes: 0..3 char (H), 4..6 glove (DWP)
    fc1c = fc1_W[:, DW:].T                          # [H, HID]
    fc1g = fc1_W[:, :DW].T                          # [DW, HID]
    for k in range(H // 128):
        fc1t[:, k, :] = fc1c[k * 128:(k + 1) * 128]
    for k in range(DWP // 128):
        blk = fc1g[k * 128:min((k + 1) * 128, DW)]
        fc1t[:blk.shape[0], H // 128 + k, :] = blk

    shared = dict(
        wp12=wp12, biasp=biasp,
        cembT=cembT.astype(NP_BF16),
        wih64=wih64.astype(NP_BF16),
        fc1t=fc1t.astype(NP_BF16),
        fc1b=np.ascontiguousarray(fc1_b.reshape(-1, 128).T),
        fc2T=np.ascontiguousarray(fc2_W.T.reshape(-1, 128, OUT).transpose(1, 0, 2)),
        fc2b=fc2_b,
    )

    # --- per-core tensors ---
    rows_per = N // NCORES
    in_maps = []
    for ci in range(NCORES):
        # lane word assignment for this core; -1 = dummy
        lane_words = []
        for bwords in bins:
            seq = []
            for (l, k) in bwords:
                wl = per_core_words[l][ci]
                seq.append((wl[k] if k < len(wl) else -1, l))
            lane_words.append(seq)

        oh = np.zeros((L, 128, NL), NP_FP8)
        mask = np.zeros(NW, NP_BF16)
        for lane, seq in enumerate(lane_words):
            r0 = 0
            for wi, (w, l) in enumerate(seq):
                if w >= 0:
                    mask[stash_of[(lane, wi)]] = 1.0
                    for t in range(l):
                        oh[r0 + t, char_indices[w, t], lane] = 1.0
                r0 += l

        # glove rows packed transposed: [128, DWP//128, 512]
        wids = word_indices[ci * rows_per:(ci + 1) * rows_per]
        gl = np.zeros((512, DWP), np.float32)
        gl[:len(wids), :DW] = glove_table[wids]
        glp = np.ascontiguousarray(
            gl.T.reshape(DWP // 128, 128, 512).transpose(1, 0, 2))

        in_maps.append(dict(
            onehot=oh,
            mask=mask,
            glp=glp.astype(NP_BF16),
            **shared,
        ))
    return in_maps


def kernel(**inputs):
    char_lengths = np.asarray(inputs["char_lengths"])
    char_indices = np.asarray(inputs["char_indices"])
    N = char_indices.shape[0]
    VW, DW = np.asarray(inputs["glove_table"]).shape
    VC, DC = np.asarray(inputs["char_embed"]).shape
    H = np.asarray(inputs["W_hh"]).shape[1]
    HID = np.asarray(inputs["fc1_W"]).shape[0]
    OUT = np.asarray(inputs["fc2_W"]).shape[0]

    sched = _build_schedule(char_lengths)
    assert sched["NL"] <= 512, f"lane count {sched['NL']} exceeds PSUM bank"

    nc = _build_program(sched["NL"], sched["NW"], sched["A"],
                        sched["switches"], sched["finals"],
                        VC, DC, H, DW, HID, OUT, N)
    in_maps = _prep_inputs(inputs, sched)

    res = None
    for attempt in range(3):
        try:
            res = run_bass_kernel_spmd(nc, in_maps, list(range(NCORES)))
            break
        except Exception:
            if attempt == 2:
                raise
            time.sleep(2.0)
    global _LAST_RESULTS
    _LAST_RESULTS = res
    return np.array(res.results[0]["out"], dtype=np.float32)


_LAST_RESULTS = None
